# revision 41
# baseline (speedup 1.0000x reference)
"""Trainium2 Bass kernel for GQA attention (B=2, S=2048, DIM=4096, 32 q-heads,
8 kv-heads, head_dim=128, RoPE, causal).

Sharding: 8 cores = (2 batches) x (4 kv-head groups of 2 kv-heads / 8 q-heads).
No collectives: each core emits a partial (wo row-sharded) output; the host
sums the 4 group-partials per batch.

Per-core layouts (everything stays "transposed", head_dim/feature on
partitions, tokens on the free axis) so no on-chip transposes are needed:
  xT  [4096, S]      QK-proj:   QT/KT = Wqk^T @ xT     (lhsT = Wqk, rhs = xT)
  V   [S, 256]       V-proj:    V = xT^T @ Wv          (lhsT = xT,  rhs = Wv)
  S^T [kv, q]        scores:    lhsT = KT tile, rhs = QT
  P^T = exp(S^T)     (scores ~ N(0,1): softmax without max subtraction)
  OUT^T [d, q]       PV:        lhsT = V tile,  rhs = P^T
  sums [1, q]        ones-matmul over P^T; normalization by 1/sums applied to
                     OUT^T via gpsimd partition_broadcast + DVE multiply.
  final^T [4096, q]  wo-proj:   lhsT = wo tile, rhs = OUT^T

v2 structure (fused pipeline):
  - j-loop fusion: projections for token-block j+1 are emitted as fill units
    interleaved into attention(j)'s inner loop, so the PE never idles on the
    exp dependency chain.
  - attention inner loop software-pipelined by one kv-tile: scores(k) issue
    while PV(k-1) consumes the exp output of the previous tile.
  - wo projection runs as a dense tail phase over the full osb, loading each
    wo Dt-tile once (8.4MB instead of 33.5MB of DMA).
  - output partial is written in bf16 (host accumulates in f32).

RoPE: wq/wk columns are permuted per head on the host (even dims first, odd
dims second) so rotation becomes the "rotate-half" form; scores are invariant
under a shared permutation of q/k head dims. 1/sqrt(HD) is folded into wq.
"""

import numpy as np
import ml_dtypes

BF16 = ml_dtypes.bfloat16

B, S_FULL, DIM = 2, 2048, 4096
NH, NKV, HD = 32, 8, 128
NREP = NH // NKV
NCORES = 8
NGRP = 4            # head groups
NQH = NH // NGRP    # 8 q heads per core
NKVH = NKV // NGRP  # 2 kv heads per core
NKT = DIM // 128    # 32 contraction tiles
TB = 512            # token block
NCT = (NQH * HD + NKVH * HD) // 128  # 10 col tiles (8 q + 2 k)

_cache = {}


def _build(S, reps=1, phase="all", **opt):
    """Build + bacc-compile the per-core Bass module (same program on all 8)."""
    import concourse.mybir as mybir
    import concourse.tile as tile
    from concourse import bacc

    f32 = mybir.dt.float32
    bf16 = mybir.dt.bfloat16
    Exp = mybir.ActivationFunctionType.Exp
    mult = mybir.AluOpType.mult

    NT = S // TB          # token blocks
    NJ = S // TB          # q blocks
    NKVT = S // 128       # kv tiles
    NDT = DIM // 128
    NCH = 4               # xt split into 4 chunk tiles (parallel DMA)
    KCH = NKT // NCH      # k-tiles per chunk
    TPB = TB // 128       # kv tiles per token block

    nc = bacc.Bacc("TRN2", target_bir_lowering=False, debug=False,
                   num_devices=NCORES)

    xt_d = nc.dram_tensor("xt", [NT, 128, NKT, TB], bf16, kind="ExternalInput")
    wqk_d = nc.dram_tensor("wqk", [NCT, 128, NKT, 128], bf16, kind="ExternalInput")
    wv_d = nc.dram_tensor("wv", [128, NKT, NKVH * HD], bf16, kind="ExternalInput")
    wo_d = nc.dram_tensor("wo", [NDT, 128, NQH, 128], bf16, kind="ExternalInput")
    cosf_d = nc.dram_tensor("cosf", [128, S], bf16, kind="ExternalInput")
    sinf_d = nc.dram_tensor("sinf", [128, S], bf16, kind="ExternalInput")
    pat_d = nc.dram_tensor("pat", [128, 4, TB], bf16, kind="ExternalInput")
    ones_d = nc.dram_tensor("ones", [128, 1], bf16, kind="ExternalInput")
    out_d = nc.dram_tensor("outp", [NDT, 128, S], bf16, kind="ExternalOutput")

    with tile.TileContext(nc) as tc:
        from contextlib import ExitStack
        with ExitStack() as ctx:
            const_p = ctx.enter_context(tc.tile_pool(name="const", bufs=1))
            xt_p = ctx.enter_context(tc.tile_pool(name="xt", bufs=6))
            wqk_p = ctx.enter_context(tc.tile_pool(name="wqk", bufs=4))
            wo_p = ctx.enter_context(tc.tile_pool(name="wo", bufs=3))
            qt_p = ctx.enter_context(tc.tile_pool(name="qt", bufs=2))
            kt_p = ctx.enter_context(tc.tile_pool(name="kt", bufs=1))
            vt_p = ctx.enter_context(tc.tile_pool(name="vt", bufs=1))
            osb_p = ctx.enter_context(tc.tile_pool(name="osb", bufs=1))
            tmp_p = ctx.enter_context(tc.tile_pool(name="tmp", bufs=2))
            pt_p = ctx.enter_context(tc.tile_pool(name="pt", bufs=opt.get("pt", 6)))
            st_p = ctx.enter_context(tc.tile_pool(name="st", bufs=3))
            rr_p = ctx.enter_context(tc.tile_pool(name="rr", bufs=2))
            rb_p = ctx.enter_context(tc.tile_pool(name="rb", bufs=2))
            psA = ctx.enter_context(tc.tile_pool(name="psA", bufs=opt.get("psA", 2), space="PSUM"))
            psP = ctx.enter_context(tc.tile_pool(name="psP", bufs=opt.get("psP", 1), space="PSUM"))
            psO = ctx.enter_context(tc.tile_pool(name="psO", bufs=opt.get("psO", 2), space="PSUM"))
            psS = ctx.enter_context(tc.tile_pool(name="psS", bufs=1, space="PSUM"))

            # constants loaded once
            cosf = const_p.tile([128, S], bf16, tag="cosf")
            sinf = const_p.tile([128, S], bf16, tag="sinf")
            pat = const_p.tile([128, 4, TB], bf16, tag="pat")
            ones = const_p.tile([128, 1], bf16, tag="ones")
            wv = const_p.tile([128, NKT, NKVH * HD], bf16, tag="wv")

            for _rep in range(reps):
                pmall = psS.tile([128, TB], f32, tag="pmall")
                kt = kt_p.tile([128, NKVH, S], bf16, tag="kt")
                vt = vt_p.tile([128, NKVT, NKVH * HD], bf16, tag="vt")
                osb = osb_p.tile([128, NQH, S], bf16, tag="osb")

                def load_consts():
                    if _rep == 0:
                        nc.sync.dma_start(ones[:], ones_d[:])
                        nc.sync.dma_start(wv[:], wv_d[:])
                        nc.sync.dma_start(cosf[:], cosf_d[:])
                        nc.sync.dma_start(sinf[:], sinf_d[:])
                        nc.sync.dma_start(pat[:], pat_d[:])

                # ---- projections for block j as a list of fill units ----
                def make_proj_units(j, qt):
                    xch = []

                    def dma_x():
                        for ch in range(NCH):
                            xc = xt_p.tile([128, KCH, TB], bf16, tag="xt")
                            nc.sync.dma_start(
                                xc[:], xt_d[j, :, ch * KCH:(ch + 1) * KCH, :])
                            xch.append(xc)

                    def ct_unit(ct):
                        def f():
                            w = wqk_p.tile([128, NKT, 128], bf16, tag="wqk")
                            KQ = NKT // 4
                            for q4 in range(4):
                                nc.sync.dma_start(
                                    w[:, q4 * KQ:(q4 + 1) * KQ, :],
                                    wqk_d[ct, :, q4 * KQ:(q4 + 1) * KQ, :])
                            ps = psP.tile([128, TB], f32, tag="pp")
                            for k in range(NKT):
                                nc.tensor.matmul(
                                    ps[:], w[:, k, :],
                                    xch[k // KCH][:, k % KCH, :],
                                    start=(k == 0), stop=(k == NKT - 1))
                            if ct < NQH:
                                dst = qt[:, ct, :]
                            else:
                                dst = kt[:, ct - NQH, j * TB:(j + 1) * TB]
                            nc.scalar.copy(dst, ps[:])
                        return f

                    def v_unit(st):
                        def f():
                            ps = psP.tile([128, NKVH * HD], f32, tag="pp")
                            for k in range(NKT):
                                nc.tensor.matmul(
                                    ps[:],
                                    xch[k // KCH][:, k % KCH,
                                                  st * 128:(st + 1) * 128],
                                    wv[:, k, :],
                                    start=(k == 0), stop=(k == NKT - 1))
                            nc.scalar.copy(vt[:, j * TPB + st, :], ps[:])
                        return f

                    def rope_unit(hs):
                        def f():
                            if hs < NQH:
                                d = qt[:, hs, :]
                            else:
                                d = kt[:, hs - NQH, j * TB:(j + 1) * TB]
                            cs = cosf[:, j * TB:(j + 1) * TB]
                            sn = sinf[:, j * TB:(j + 1) * TB]
                            tmp = tmp_p.tile([128, TB], bf16, tag="tmp")
                            nc.vector.tensor_copy(tmp[0:64, :], d[64:128, :])
                            nc.vector.tensor_copy(tmp[64:128, :], d[0:64, :])
                            nc.vector.tensor_tensor(d, d, cs, mult)
                            nc.vector.tensor_tensor(tmp[:], tmp[:], sn, mult)
                            nc.vector.tensor_add(d, d, tmp[:])
                        return f

                    units = [dma_x]
                    units += [ct_unit(ct) for ct in range(NCT)]
                    units += [v_unit(st) for st in range(TPB)]
                    units += [rope_unit(hs) for hs in range(NQH + NKVH)]
                    return units

                # ---- attention for q block j, with fill units interleaved ----
                def attention(j, qt, units):
                    nun = len(units)
                    issued = [0]
                    nslots = (NQH // 2) * ((j + 1) * TPB)
                    slot = [0]

                    def pace():
                        slot[0] += 1
                        target = (nun * slot[0] + nslots - 1) // nslots
                        while issued[0] < min(target, nun):
                            units[issued[0]]()
                            issued[0] += 1

                    nkv = (j + 1) * TPB
                    for pi in range(NQH // 2):
                        kvh = (2 * pi) // NREP
                        hh = (2 * pi, 2 * pi + 1)
                        po = [psO.tile([128, TB], f32, tag="po",
                                       name=f"po{pi}_{i}") for i in range(2)]

                        def emit_pv(prev):
                            pts, off, N, k = prev
                            for i in range(2):
                                nc.tensor.matmul(
                                    po[i][:, off:],
                                    vt[:, k, kvh * HD:(kvh + 1) * HD],
                                    pts[i][:, :N],
                                    start=(k == 0), stop=(k == nkv - 1),
                                    skip_group_check=True)
                            if phase == "noones":
                                return
                            for i in range(2):
                                nc.tensor.matmul(
                                    pmall[64 * i:64 * i + 1, off:],
                                    ones[:], pts[i][:, :N],
                                    start=(k == 0), stop=(k == nkv - 1),
                                    tile_position=(0, 64 * i),
                                    skip_group_check=True)

                        pending = []
                        for k in range(nkv):
                            kd = k - j * TPB
                            off = max(0, kd) * 128   # causal col offset
                            N = TB - off
                            ss = psA.tile([128, 2, TB], f32, tag="ps")
                            for i, h in enumerate(hh):
                                nc.tensor.matmul(
                                    ss[:, i, :N],
                                    kt[:, kvh, k * 128:(k + 1) * 128],
                                    qt[:, h, off:],
                                    start=True, stop=True)
                            pt2 = pt_p.tile([128, 2, TB], bf16, tag="pt")
                            nc.scalar.activation(pt2[:, :, :N], ss[:, :, :N],
                                                 Exp)
                            if kd >= 0:
                                for i in range(2):
                                    nc.vector.tensor_tensor(
                                        pt2[:, i, :N], pt2[:, i, :N],
                                        pat[:, kd, off:], mult)
                            pts = [pt2[:, 0, :], pt2[:, 1, :]]
                            pending.append((pts, off, N, k))
                            if len(pending) > opt.get("depth", 3):
                                emit_pv(pending.pop(0))
                            pace()
                        for prev in pending:
                            emit_pv(prev)

                        for i, h in enumerate(hh):
                            dst = osb[:, h, j * TB:(j + 1) * TB]
                            # copy raw PV out first: frees the PSUM bank
                            # without waiting on the normalization chain
                            nc.scalar.copy(dst, po[i][:])
                            if phase == "noones":
                                continue
                            rr = rr_p.tile([1, TB], f32, tag="rr")
                            nc.vector.reciprocal(
                                rr[:], pmall[64 * i:64 * i + 1, :])
                            rb = rb_p.tile([128, TB], f32, tag="rb")
                            nc.gpsimd.partition_broadcast(rb[:], rr[:])
                            nc.vector.tensor_tensor(dst, dst, rb[:], mult)
                    # any leftover fill units (shouldn't happen)
                    while issued[0] < nun:
                        units[issued[0]]()
                        issued[0] += 1

                # ---- wo tail: dense, each wo Dt-tile loaded once ----
                def wo_tail():
                    for Dt in range(NDT):
                        wo = wo_p.tile([128, NQH, 128], bf16, tag="wo",
                                       name=f"wo{Dt}")
                        nc.sync.dma_start(wo[:, 0:NQH // 2, :],
                                          wo_d[Dt, :, 0:NQH // 2, :])
                        nc.sync.dma_start(wo[:, NQH // 2:, :],
                                          wo_d[Dt, :, NQH // 2:, :])
                        for jp in range(NJ // 2):
                            pw = psA.tile([128, 2, TB], f32, tag="ps",
                                          name=f"pw{Dt}_{jp}")
                            for i in range(2):
                                jw = 2 * jp + i
                                for dt in range(NQH):
                                    nc.tensor.matmul(
                                        pw[:, i, :], wo[:, dt, :],
                                        osb[:, dt, jw * TB:(jw + 1) * TB],
                                        start=(dt == 0), stop=(dt == NQH - 1))
                            stg = st_p.tile([128, 2, TB], bf16, tag="st",
                                            name=f"st{Dt}_{jp}")
                            nc.vector.tensor_copy(stg[:], pw[:])
                            nc.sync.dma_start(
                                out_d[Dt, :, jp * 2 * TB:(jp + 1) * 2 * TB],
                                stg[:, :, :])

                # ---- main fused loop ----
                qt_next = qt_p.tile([128, NQH, TB], bf16, tag="qt")
                units0 = make_proj_units(0, qt_next)
                if phase == "p1":
                    for j in range(NT):
                        if j > 0:
                            qt_next = qt_p.tile([128, NQH, TB], bf16, tag="qt")
                            units0 = make_proj_units(j, qt_next)
                        for ui, u in enumerate(units0):
                            u()
                            if j == 0 and ui == 2:
                                load_consts()
                    continue
                for ui, u in enumerate(units0):
                    u()
                    if ui == 2:
                        load_consts()
                for j in range(NJ):
                    qt_cur = qt_next
                    if j + 1 < NJ:
                        qt_next = qt_p.tile([128, NQH, TB], bf16, tag="qt")
                        units = make_proj_units(j + 1, qt_next)
                    else:
                        units = []
                    attention(j, qt_cur, units)
                if phase != "nowo":
                    wo_tail()

    nc.compile()
    return nc


_PERM = None


def _prep_core_inputs(x, freqs_cis, mask, wq, wk, wv, wo, b, g, S):
    """Host-side shard/permute/prepack for core (batch b, group g)."""
    global _PERM
    if _PERM is None or len(_PERM) != HD:
        _PERM = np.concatenate([np.arange(0, HD, 2), np.arange(1, HD, 2)])
    perm = _PERM
    NT = S // TB

    qh0 = g * NQH            # first q head
    kh0 = g * NKVH           # first kv head

    wq_g = wq[:, qh0 * HD:(qh0 + NQH) * HD].reshape(DIM, NQH, HD)[:, :, perm]
    wq_g = (wq_g * np.float32(HD ** -0.5)).reshape(DIM, NQH * HD)
    wk_g = wk[:, kh0 * HD:(kh0 + NKVH) * HD].reshape(DIM, NKVH, HD)[:, :, perm]
    wk_g = wk_g.reshape(DIM, NKVH * HD)
    wqk = np.concatenate([wq_g, wk_g], axis=1)          # [DIM, 1280]
    wqk = np.ascontiguousarray(
        wqk.reshape(NKT, 128, NCT, 128).transpose(2, 1, 0, 3)).astype(BF16)

    wv_g = wv[:, kh0 * HD:(kh0 + NKVH) * HD]            # [DIM, 256]
    wv_g = np.ascontiguousarray(
        wv_g.reshape(NKT, 128, NKVH * HD).transpose(1, 0, 2)).astype(BF16)

    wo_g = wo[qh0 * HD:(qh0 + NQH) * HD, :]             # [1024, DIM]
    wo_g = np.ascontiguousarray(
        wo_g.reshape(NQH, 128, DIM // 128, 128).transpose(2, 1, 0, 3)).astype(BF16)

    xb = x[b, :S, :]                                    # [S, DIM]
    xt = np.ascontiguousarray(
        xb.reshape(NT, TB, NKT, 128).transpose(0, 3, 2, 1)).astype(BF16)

    cos = freqs_cis[:S, :, 0]                           # [S, 64]
    sin = freqs_cis[:S, :, 1]
    cosf = np.ascontiguousarray(np.concatenate([cos, cos], 1).T).astype(BF16)
    sinf = np.ascontiguousarray(np.concatenate([-sin, sin], 1).T).astype(BF16)

    sub = mask[:TB, :TB]                                # [q, kv]
    pat = (sub.T.reshape(4, 128, TB) >= -0.5).astype(BF16)
    pat = np.ascontiguousarray(pat.transpose(1, 0, 2))

    ones = np.ones((128, 1), dtype=BF16)
    return {"xt": xt, "wqk": wqk, "wv": wv_g, "wo": wo_g,
            "cosf": cosf, "sinf": sinf, "pat": pat, "ones": ones}


def run(x, freqs_cis, mask, wq, wk, wv, wo, S=S_FULL, reps=1, time_it=False):
    from concourse.bass_utils import run_bass_kernel_spmd

    key = (S, reps)
    if key not in _cache:
        _cache[key] = _build(S, reps)
    nc = _cache[key]

    in_maps = []
    for c in range(NCORES):
        b, g = c // NGRP, c % NGRP
        in_maps.append(_prep_core_inputs(x, freqs_cis, mask, wq, wk, wv, wo,
                                         b, g, S))
    res = run_bass_kernel_spmd(nc, in_maps, core_ids=list(range(NCORES)))

    out = np.zeros((B, S, DIM), dtype=np.float32)
    for c in range(NCORES):
        b = c // NGRP
        pt = res.results[c]["outp"].astype(np.float32).reshape(DIM, S)
        out[b] += pt.T
    return out


def kernel(x, start_pos, freqs_cis, mask, wq, wk, wv, wo):
    x = np.asarray(x, dtype=np.float32)
    freqs_cis = np.asarray(freqs_cis, dtype=np.float32)
    mask = np.asarray(mask, dtype=np.float32)
    wq = np.asarray(wq, dtype=np.float32)
    wk = np.asarray(wk, dtype=np.float32)
    wv = np.asarray(wv, dtype=np.float32)
    wo = np.asarray(wo, dtype=np.float32)
    return run(x, freqs_cis, mask, wq, wk, wv, wo, S=x.shape[1], reps=1)
